# revision 4
# baseline (speedup 1.0000x reference)
"""DeformAlign Trainium2 kernel v4 (one image per NeuronCore, 8-way batch parallel).

Baseline pipeline, restructured so dma_gathers start ~290us instead of ~550us:
  conv1 half A -> conv2 half A (row-groups via PSUM, offT transposed on the
  fly; offsets never materialize) -> fields half A -> wrp half A -> gathers
  for chunks 0..1 begin while conv1 half B / conv2 half B / fields half B
  run on PE/DVE under the gather window.
Combine uses one 4-corner multiply (5-dim AP) + 2 adds per (tap, chunk).
Weights host-packed fp16; x/ref passed fp16; output staged fp16.
"""
import contextlib
import numpy as np

import concourse.bass as bass
import concourse.bacc as bacc
import concourse.mybir as mybir
from concourse.tile import TileContext

F32 = mybir.dt.float32
F16 = mybir.dt.float16
I16 = mybir.dt.int16
AL = mybir.AluOpType
AF = mybir.ActivationFunctionType

H = W = 128
C = 64
O = 64
HW = H * W
PAD = 130
PHW = PAD * PAD
RPC = 131            # table records per x-column
NRECT = 16900        # table records
RECW = 128           # fp16 elements per record
ELEM = 256           # gather elem: 2 vertically-adjacent records
NCHUNK = 16
CH_Y = 8
CH_PIX = CH_Y * W    # 1024
NIDXC = 9 * CH_PIX   # 9216 indices per chunk
NCKY = NCHUNK * 9 * CH_Y  # 1152
NW = 576             # idx cols per chunk in wrp
GSZ = 768
CCH = 512
P0_FIRST = PAD + 1
P_END = PAD * (H + 1) - 1
HROWS = 66           # t1 rows per half tile
HLEN = PAD * HROWS


def bcast_ap(sliced, n):
    return bass.AP(sliced.tensor, sliced.offset, list(sliced.ap) + [[0, n]])


def build_nc():
    nc = bacc.Bacc("TRN2", target_bir_lowering=False)

    x_in = nc.dram_tensor("x", [C, HW], F16, kind="ExternalInput")
    ref_in = nc.dram_tensor("ref", [C, PHW], F16, kind="ExternalInput")
    w1l_in = nc.dram_tensor("w1l", [128, 3 * 64], F16, kind="ExternalInput")
    w1s_in = nc.dram_tensor("w1s", [64, 3 * 64], F16, kind="ExternalInput")
    w2l_in = nc.dram_tensor("w2l", [128, 3 * 18], F16, kind="ExternalInput")
    w2s_in = nc.dram_tensor("w2s", [64, 3 * 18], F16, kind="ExternalInput")
    wel_in = nc.dram_tensor("wel", [128, 4 * 64], F16, kind="ExternalInput")
    wes_in = nc.dram_tensor("wes", [64, 64], F16, kind="ExternalInput")
    b1_in = nc.dram_tensor("b1", [64], F32, kind="ExternalInput")
    b2_in = nc.dram_tensor("b2", [18], F32, kind="ExternalInput")
    ident_in = nc.dram_tensor("ident", [128, 128], F16, kind="ExternalInput")
    yb_in = nc.dram_tensor("ybias", [128, 3 * 128], F16, kind="ExternalInput")
    xb_in = nc.dram_tensor("xbias", [128, 3 * 128], F16, kind="ExternalInput")

    out_t = nc.dram_tensor("out", [O, HW], F16, kind="ExternalOutput")

    with TileContext(nc) as tc, contextlib.ExitStack() as ctx:
        pool = ctx.enter_context(tc.tile_pool(name="sb", bufs=1))
        bigp = ctx.enter_context(tc.tile_pool(name="bg", bufs=3))
        wpp = ctx.enter_context(tc.tile_pool(name="wp", bufs=2))
        rpool = ctx.enter_context(tc.tile_pool(name="rb", bufs=2))
        ppool = ctx.enter_context(tc.tile_pool(name="ps", bufs=2, space="PSUM"))
        peins = ctx.enter_context(tc.tile_pool(name="pe", bufs=2, space="PSUM"))
        dpool = ctx.enter_context(tc.tile_pool(name="dr", bufs=1, space="DRAM"))

        # ---------------- constants ----------------
        ident = pool.tile([128, 128], F16)
        nc.sync.dma_start(ident[:], ident_in[:])
        ybias = pool.tile([128, 3, 128], F16)
        nc.sync.dma_start(ybias[:].rearrange("p a b -> p (a b)"), yb_in[:])
        xbias = pool.tile([128, 3, 128], F16)
        nc.sync.dma_start(xbias[:].rearrange("p a b -> p (a b)"), xb_in[:])
        b1t = pool.tile([64, 1], F32)
        nc.sync.dma_start(b1t[:], bass.AP(b1_in, 0, [[1, 64], [1, 1]]))
        b2t = pool.tile([18, 1], F32)
        nc.sync.dma_start(b2t[:], bass.AP(b2_in, 0, [[1, 18], [1, 1]]))
        w1l = pool.tile([128, 3, 64], F16)
        nc.sync.dma_start(w1l[:].rearrange("p a b -> p (a b)"), w1l_in[:])
        w1s = pool.tile([64, 3, 64], F16)
        nc.sync.dma_start(w1s[:].rearrange("p a b -> p (a b)"), w1s_in[:])
        w2l = pool.tile([128, 3, 18], F16)
        nc.scalar.dma_start(w2l[:].rearrange("p a b -> p (a b)"), w2l_in[:])
        w2s = pool.tile([64, 3, 18], F16)
        nc.scalar.dma_start(w2s[:].rearrange("p a b -> p (a b)"), w2s_in[:])
        wel = pool.tile([128, 4, 64], F16)
        nc.sync.dma_start(wel[:].rearrange("p a b -> p (a b)"), wel_in[:])
        wes = pool.tile([64, 1, 64], F16)
        nc.scalar.dma_start(wes[:].rearrange("p a b -> p (a b)"), wes_in[:])

        # token table in DRAM + border zeros
        table = dpool.tile([NRECT, RECW], F16)
        ztile = pool.tile([128, 192], F16)
        nc.vector.memset(ztile[:], 0.0)
        nc.sync.dma_start(bass.AP(table.tensor, table.offset, [[RECW, 128], [1, 64]]),
                          ztile[:, 0:64])
        nc.sync.dma_start(bass.AP(table.tensor, table.offset + 128 * RECW, [[RECW, 3], [1, 64]]),
                          ztile[0:3, 0:64])
        nc.sync.dma_start(bass.AP(table.tensor, table.offset + 128 * RPC * RECW + 64,
                                  [[RECW, 128], [1, 64]]),
                          ztile[:, 0:64])
        nc.sync.dma_start(bass.AP(table.tensor, table.offset + (128 * RPC + 128) * RECW + 64,
                                  [[RECW, 3], [1, 64]]),
                          ztile[0:3, 0:64])
        for yr in (0, 129, 130):
            nc.sync.dma_start(bass.AP(table.tensor, table.offset + yr * RECW,
                                      [[RPC * RECW, 128], [1, RECW]]),
                              ztile[:, 0:128])
            nc.sync.dma_start(bass.AP(table.tensor, table.offset + (128 * RPC + yr) * RECW,
                                      [[1, 1], [1, RECW]]),
                              ztile[0:1, 0:128])

        # ---------------- conv1 half A ----------------
        refdup = bigp.tile([128, PHW + 4], F16, tag="big")
        nc.vector.memset(refdup[0:64, PHW:PHW + 4], 0.0)
        nc.vector.memset(refdup[64:128, PHW - 1:PHW + 4], 0.0)
        nc.sync.dma_start(refdup[0:64, 0:PHW], ref_in[:])
        nc.scalar.dma_start(refdup[64:128, 0:PHW - 1],
                            bass.AP(ref_in, 1, [[PHW, 64], [1, PHW - 1]]))

        t1p = bigp.tile([64, PHW], F16, tag="big")

        def conv1_sweep(pstart, pend):
            for j in range((pend - pstart + CCH - 1) // CCH):
                p0 = pstart + j * CCH
                n = min(CCH, pend - p0)
                ps = ppool.tile([128, CCH], F32, tag="tp")
                for ki in range(3):
                    d0 = (ki - 1) * PAD - 1
                    nc.tensor.matmul(ps[0:64, 0:n], w1l[:, ki, :],
                                     refdup[:, p0 + d0: p0 + d0 + n],
                                     start=(ki == 0), stop=False)
                    nc.tensor.matmul(ps[0:64, 0:n], w1s[:, ki, :],
                                     refdup[0:64, p0 + d0 + 2: p0 + d0 + 2 + n],
                                     start=False, stop=(ki == 2))
                nc.scalar.activation(t1p[:, p0:p0 + n], ps[0:64, 0:n], AF.Relu, bias=b1t[:])

        t1v = t1p[:].rearrange("c (y x) -> c y x", y=PAD)
        nc.vector.memset(t1v[:, 0:1, :], 0.0)
        nc.vector.memset(t1v[:, 129:130, :], 0.0)
        conv1_sweep(P0_FIRST, PAD * HROWS)
        nc.vector.memset(t1v[:, 0:HROWS, 0:1], 0.0)
        nc.vector.memset(t1v[:, 0:HROWS, 129:130], 0.0)

        # t1d half A (t1 rows 0..65, duplicated/shifted)
        t1dA = bigp.tile([128, HLEN + 4], F16, tag="big")
        nc.vector.memset(t1dA[0:64, HLEN:HLEN + 4], 0.0)
        nc.vector.memset(t1dA[64:128, HLEN - 1:HLEN + 4], 0.0)
        nc.sync.dma_start(t1dA[0:64, 0:HLEN], t1p[:, 0:HLEN])
        nc.scalar.dma_start(t1dA[64:128, 0:HLEN - 1], t1p[:, 1:HLEN])

        # ---------------- conv2 (row groups, PSUM -> offT on the fly) -------
        offTa = pool.tile([128, 64, 18], F16)
        offTb = pool.tile([128, 64, 18], F16)

        def conv2_half(t1d_t, base_row, offT_t):
            # output rows r in [base_row, base_row+64), t1d_t holds padded t1
            # rows base_row..base_row+65 at offset 0
            r = 0
            while r < 64:
                n = min(3, 64 - r)
                L = PAD * n - 2
                p0 = PAD * (r + 1) + 1
                ps2 = ppool.tile([18, 390], F32, tag="tp")
                for ki in range(3):
                    d0 = (ki - 1) * PAD - 1
                    nc.tensor.matmul(ps2[:, 0:L], w2l[:, ki, :],
                                     t1d_t[:, p0 + d0: p0 + d0 + L],
                                     start=(ki == 0), stop=False)
                    nc.tensor.matmul(ps2[:, 0:L], w2s[:, ki, :],
                                     t1d_t[0:64, p0 + d0 + 2: p0 + d0 + 2 + L],
                                     start=False, stop=(ki == 2))
                tr2 = wpp.tile([18, 390], F16, tag="tr2")
                nc.scalar.activation(tr2[:, 0:L], ps2[:, 0:L], AF.Identity, bias=b2t[:])
                tpo = ppool.tile([128, 54], F16, tag="tph")
                for j in range(n):
                    nc.tensor.transpose(tpo[:, j * 18:(j + 1) * 18],
                                        tr2[:, j * PAD: j * PAD + 128],
                                        ident[0:18, 0:18])
                nc.scalar.activation(offT_t[:, r:r + n, :], tpo[:, 0:n * 18], AF.Copy)
                r += n

        conv2_half(t1dA, 0, offTa)

        # ---------------- x blocks -> token table interior ----------------
        for blk in range(16):
            xhb = wpp.tile([64, 1024], F16, tag="xhb")
            nc.sync.dma_start(xhb[:], x_in[:, blk * 1024:(blk + 1) * 1024])
            tps = ppool.tile([128, 512], F16, tag="tph")
            tps2 = ppool.tile([128, 512], F16, tag="tph")
            for j in range(8):
                nc.tensor.transpose(tps[:, j * C:(j + 1) * C],
                                    xhb[:, j * W:(j + 1) * W],
                                    ident[0:64, 0:64])
                nc.tensor.transpose(tps2[0:127, j * C:(j + 1) * C],
                                    xhb[:, j * W + 1:(j + 1) * W],
                                    ident[0:64, 0:64])
            xib = wpp.tile([128, 8, 128], F16, tag="xib")
            nc.vector.memset(xib[96:128, :, 64:128], 0.0)
            nc.scalar.activation(xib[:, :, 0:64], tps[:], AF.Copy)
            nc.scalar.activation(xib[0:127, :, 64:128], tps2[0:127, :], AF.Copy)
            # cx=0 column slot1 = x col 0 (these 8 yrows)
            nc.sync.dma_start(
                bass.AP(table.tensor, table.offset + (1 + 8 * blk) * RECW + 64,
                        [[1, 1], [RECW, 8], [1, C]]),
                xib[0:1, :, 0:64])
            # interior records
            nc.sync.dma_start(
                bass.AP(table.tensor, table.offset + (RPC + 1 + 8 * blk) * RECW,
                        [[RPC * RECW, 128], [1, 8 * RECW]]),
                xib[:].rearrange("p y c -> p (y c)"))

        # ---------------- fields ----------------
        wf = pool.tile([128, 4, 9, H], F16)
        qi = pool.tile([128, NCHUNK, 9, CH_Y], I16)
        qfold = pool.tile([16, 8, 576], I16)
        wrp = pool.tile([128, NCKY * 8], I16)
        tmpa = pool.tile([128, 64], F32)
        tmpb = pool.tile([128, 64], F32)
        yw0 = pool.tile([128, 64], F32)
        yw1 = pool.tile([128, 64], F32)
        xw0 = pool.tile([128, 64], F32)
        xw1 = pool.tile([128, 64], F32)
        vm = pool.tile([128, 64], F32)
        qf = pool.tile([128, 64], F32)

        def fields_half(h, offT_t):
            y0 = 64 * h
            for k in range(9):
                ki, kj = k // 3, k % 3
                for axis in range(2):
                    ob = offT_t[:, :, 2 * k + axis]
                    bias = (ybias[:, ki, y0:y0 + 64] if axis == 0
                            else xbias[:, kj, 0:64])
                    w0, w1_ = (yw0, yw1) if axis == 0 else (xw0, xw1)
                    nc.vector.tensor_tensor(tmpa[:], ob, bias, op=AL.add)
                    nc.vector.tensor_scalar(tmpb[:], tmpa[:], 12582912.0, -12582912.0,
                                            op0=AL.add, op1=AL.add)
                    nc.vector.tensor_tensor(vm[:], tmpb[:], tmpa[:], op=AL.is_gt)
                    nc.vector.tensor_tensor(tmpb[:], tmpb[:], vm[:], op=AL.subtract)
                    nc.vector.tensor_tensor(w1_[:], tmpa[:], tmpb[:], op=AL.subtract)
                    nc.vector.tensor_scalar(vm[:], tmpb[:], 8.0, None, op0=AL.is_ge)
                    nc.vector.tensor_scalar(tmpa[:], tmpb[:], 135.0, None, op0=AL.is_le)
                    nc.vector.tensor_tensor(vm[:], vm[:], tmpa[:], op=AL.mult)
                    nc.vector.tensor_scalar(w0[:], w1_[:], -1.0, 1.0, op0=AL.mult, op1=AL.add)
                    nc.vector.tensor_tensor(w0[:], w0[:], vm[:], op=AL.mult)
                    nc.vector.tensor_scalar(vm[:], tmpb[:], 7.0, None, op0=AL.is_ge)
                    nc.vector.tensor_scalar(tmpa[:], tmpb[:], 134.0, None, op0=AL.is_le)
                    nc.vector.tensor_tensor(vm[:], vm[:], tmpa[:], op=AL.mult)
                    nc.vector.tensor_tensor(w1_[:], w1_[:], vm[:], op=AL.mult)
                    nc.vector.tensor_scalar(tmpa[:], tmpb[:], 7.0, 135.0, op0=AL.max, op1=AL.min)
                    if axis == 0:
                        nc.vector.tensor_copy(qf[:], tmpa[:])
                    else:
                        nc.vector.tensor_scalar(tmpa[:], tmpa[:], 131.0, -924.0,
                                                op0=AL.mult, op1=AL.add)
                        nc.vector.tensor_tensor(qf[:], qf[:], tmpa[:], op=AL.add)
                nc.vector.tensor_copy(qi[:, 8 * h:8 * h + 8, k, :],
                                      qf[:].rearrange("p (c y) -> p c y", c=8))
                nc.vector.tensor_tensor(wf[:, 0, k, y0:y0 + 64], yw0[:], xw0[:], op=AL.mult)
                nc.vector.tensor_tensor(wf[:, 1, k, y0:y0 + 64], yw0[:], xw1[:], op=AL.mult)
                nc.vector.tensor_tensor(wf[:, 2, k, y0:y0 + 64], yw1[:], xw0[:], op=AL.mult)
                nc.vector.tensor_tensor(wf[:, 3, k, y0:y0 + 64], yw1[:], xw1[:], op=AL.mult)

        def wrp_half(h):
            for ph in range(8):
                eng = [nc.sync, nc.scalar][ph % 2]
                eng.dma_start(
                    qfold[:, ph, :],
                    qi[16 * ph:16 * (ph + 1), 8 * h:8 * h + 8, :, :].rearrange(
                        "p c k y -> p (c k y)"))
            nc.vector.tensor_copy(
                wrp[0:16, 4608 * h:4608 * h + 4608].rearrange("p (s f) -> p f s", f=8),
                qfold[:])
            sl = slice(4608 * h, 4608 * h + 4608)
            nc.sync.dma_start(wrp[16:32, sl], wrp[0:16, sl])
            nc.scalar.dma_start(wrp[32:64, sl], wrp[0:32, sl])
            nc.sync.dma_start(wrp[64:128, sl], wrp[0:64, sl])

        fields_half(0, offTa)
        wrp_half(0)

        # ---------------- conv1 half B + conv2 half B (PE continues) -------
        conv1_sweep(PAD * HROWS, P_END)
        nc.vector.memset(t1v[:, HROWS:PAD, 0:1], 0.0)
        nc.vector.memset(t1v[:, HROWS:PAD, 129:130], 0.0)
        t1dB = bigp.tile([128, HLEN + 4], F16, tag="big")
        nc.vector.memset(t1dB[0:64, HLEN:HLEN + 4], 0.0)
        nc.vector.memset(t1dB[64:128, HLEN - 1:HLEN + 4], 0.0)
        nc.sync.dma_start(t1dB[0:64, 0:HLEN], t1p[:, PAD * 64:PAD * 130])
        nc.scalar.dma_start(t1dB[64:128, 0:HLEN - 1], t1p[:, PAD * 64 + 1:PAD * 130])
        conv2_half(t1dB, 64, offTb)

        # ---------------- gather / combine / transpose / einsum ----------------
        table_ap = bass.AP(table.tensor, table.offset, [[RECW, NRECT - 2], [1, ELEM]])

        def do_chunk(cnk):
            g = bigp.tile([128, NIDXC // 128, ELEM], F16, tag="big")
            for sub in range(NIDXC // GSZ):
                nc.gpsimd.dma_gather(
                    g[:, sub * (GSZ // 128):(sub + 1) * (GSZ // 128), :], table_ap,
                    wrp[:, cnk * NW + sub * (GSZ // 16):
                        cnk * NW + (sub + 1) * (GSZ // 16)],
                    GSZ, GSZ, ELEM, elem_step=RECW)

            s = rpool.tile([128, 9 * CH_Y, C], F16, tag="s")
            m4 = rpool.tile([128, 4, CH_Y, C], F16, tag="m", bufs=1)
            t2 = rpool.tile([128, 2, CH_Y, C], F16, tag="t2", bufs=1)
            for k in range(9):
                sk = s[:, k * CH_Y:(k + 1) * CH_Y, :]
                gsl = g[:, k * CH_Y:(k + 1) * CH_Y, :]
                gco4 = bass.AP(gsl.tensor, gsl.offset,
                               [gsl.ap[0], [128, 2], [64, 2], [256, CH_Y], [1, C]])
                wsl = wf[:, 0, k, cnk * CH_Y:(cnk + 1) * CH_Y]
                wco4 = bass.AP(wsl.tensor, wsl.offset,
                               [wsl.ap[0], [9 * 128, 4], [1, CH_Y], [0, C]])
                nc.vector.tensor_tensor(m4[:], gco4, wco4, op=AL.mult)
                nc.vector.tensor_tensor(
                    t2[:], m4[:, 0:2, :, :], m4[:, 2:4, :, :], op=AL.add)
                nc.vector.tensor_tensor(sk, t2[:, 0, :, :], t2[:, 1, :, :], op=AL.add)

            rhs_p = [rpool.tile([128, CH_PIX], F16, tag=f"rp{i}", name=f"rhs_p{i}_{cnk}",
                                bufs=1) for i in range(4)]
            rhs_s = rpool.tile([64, CH_PIX], F16, tag="rs", bufs=1)
            for k in range(9):
                tps = ppool.tile([128, 512], F16, tag="tph")
                for pr in range(4):
                    nc.tensor.transpose(
                        tps[:, pr * 128:(pr + 1) * 128],
                        s[:, k * CH_Y + pr * 2:k * CH_Y + pr * 2 + 2, :].rearrange(
                            "p a c -> p (a c)"),
                        ident[:, :])
                dst, prow = (rhs_s, 0) if k == 8 else (rhs_p[k // 2], 64 * (k % 2))
                dv = dst[prow:prow + 64, :].rearrange("c (pp j x) -> c pp j x",
                                                      pp=4, j=2, x=W)
                for jr in range(2):
                    nc.scalar.activation(dv[:, :, jr, :],
                                         tps[jr * 64:(jr + 1) * 64, :], AF.Copy)

            pso = peins.tile([64, CH_PIX], F32, tag="eo")
            for half in range(2):
                colr = slice(half * 512, (half + 1) * 512)
                for pr in range(4):
                    nc.tensor.matmul(pso[:, colr], wel[:, pr, :], rhs_p[pr][:, colr],
                                     start=(pr == 0), stop=False)
                nc.tensor.matmul(pso[:, colr], wes[:, 0, :], rhs_s[:, colr],
                                 start=False, stop=True)
            ost = rpool.tile([64, CH_PIX], F16, tag="os")
            nc.scalar.activation(ost[:], pso[:], AF.Copy)
            nc.sync.dma_start(out_t[:, cnk * CH_PIX:(cnk + 1) * CH_PIX], ost[:])

        do_chunk(0)
        do_chunk(1)
        fields_half(1, offTb)
        wrp_half(1)
        for cnk in range(2, NCHUNK):
            do_chunk(cnk)

    nc.compile()
    return nc


# ----------------------------------------------------------------------------
# host-side wrapper
# ----------------------------------------------------------------------------
def host_consts():
    ident = np.eye(128, dtype=np.float16)
    yb = np.zeros((128, 3, 128), np.float16)
    xb = np.zeros((128, 3, 128), np.float16)
    for j in range(3):
        yb[:, j, :] = (np.arange(128) + (j - 1) + 8).astype(np.float16)[None, :]
        xb[:, j, :] = (np.arange(128) + (j - 1) + 8).astype(np.float16)[:, None]
    return ident, yb, xb


def pack_weights(w1, w2, weight):
    w1 = np.asarray(w1, np.float32)
    w2 = np.asarray(w2, np.float32)
    wt = np.asarray(weight, np.float32).reshape(O, C, 9)
    w1l = np.zeros((128, 3, 64), np.float16)
    w1s = np.zeros((64, 3, 64), np.float16)
    w2l = np.zeros((128, 3, 18), np.float16)
    w2s = np.zeros((64, 3, 18), np.float16)
    for ki in range(3):
        for krel in range(2):
            w1l[64 * krel:64 * (krel + 1), ki, :] = w1[:, :, ki, krel].T
            w2l[64 * krel:64 * (krel + 1), ki, :] = w2[:, :, ki, krel].T
        w1s[:, ki, :] = w1[:, :, ki, 2].T
        w2s[:, ki, :] = w2[:, :, ki, 2].T
    wel = np.zeros((128, 4, 64), np.float16)
    for pr in range(4):
        wel[0:64, pr, :] = wt[:, :, 2 * pr].T
        wel[64:128, pr, :] = wt[:, :, 2 * pr + 1].T
    wes = np.ascontiguousarray(wt[:, :, 8].T.astype(np.float16))
    return w1l, w1s, w2l, w2s, wel, wes


def _pad_ref(ref_img):
    rp = np.zeros((C, PAD, PAD), np.float16)
    rp[:, 1:129, 1:129] = np.asarray(ref_img, np.float32).reshape(C, H, W)
    return np.ascontiguousarray(rp.reshape(C, PHW))


def make_in_map(xb_img, ref_img, b1, b2, packed):
    ident, yb, xbias = host_consts()
    w1l, w1s, w2l, w2s, wel, wes = packed
    return {
        "x": np.ascontiguousarray(xb_img.reshape(C, HW), np.float16),
        "ref": _pad_ref(ref_img),
        "w1l": np.ascontiguousarray(w1l.reshape(128, -1)),
        "w1s": np.ascontiguousarray(w1s.reshape(64, -1)),
        "w2l": np.ascontiguousarray(w2l.reshape(128, -1)),
        "w2s": np.ascontiguousarray(w2s.reshape(64, -1)),
        "wel": np.ascontiguousarray(wel.reshape(128, -1)),
        "wes": np.ascontiguousarray(wes),
        "b1": np.ascontiguousarray(b1, np.float32),
        "b2": np.ascontiguousarray(b2, np.float32),
        "ident": ident,
        "ybias": np.ascontiguousarray(yb.reshape(128, -1)),
        "xbias": np.ascontiguousarray(xbias.reshape(128, -1)),
    }


_NC_CACHE = None
TRACE = False
LAST_EXEC_NS = None


def kernel(x, ref_feature, w1, b1, w2, b2, weight):
    """Full-input entry point: shard batch across 8 cores, gather output."""
    global _NC_CACHE, LAST_EXEC_NS
    from concourse.bass_utils import run_bass_kernel_spmd
    x = np.asarray(x)
    ref_feature = np.asarray(ref_feature)
    B = x.shape[0]
    assert B == 8
    if _NC_CACHE is None:
        _NC_CACHE = build_nc()
    nc = _NC_CACHE
    packed = pack_weights(w1, w2, weight)
    in_maps = [make_in_map(x[b], ref_feature[b], b1, b2, packed)
               for b in range(B)]
    res = run_bass_kernel_spmd(nc, in_maps, core_ids=list(range(8)), trace=TRACE)
    LAST_EXEC_NS = res.exec_time_ns
    out = np.stack([res.results[b]["out"].reshape(O, H, W) for b in range(B)])
    return out.astype(np.float32)
